# revision 7
# baseline (speedup 1.0000x reference)
"""Trainium2 Bass kernel: row-softmax + embedding gather (batched lookup).

reference:
    probs = softmax(poi_freq_matrix, axis=1)        # [100000, 168] f32
    out   = probs[inputs_wekn]                      # [1024, 200, 168] f32

Strategy (8 NeuronCores, data-parallel over batch; each core owns 128
batch rows = 128 SBUF partitions x 200 seq positions). All positions go
through quad dma_gather: the table is packed into [25000, 4x192] bf16
"quad" rows so int16 indices (wekn//4) cover all 100000 rows; sub-row
wekn%4 is selected in place by 3 predicated copies (DVE, bf16).

The binding resource is GpSimd (Pool): the gather ucode emits
descriptors at ~7.5ns/index, so 25600 lookups/core ≈ 192us of Pool no
matter how they're batched. Everything else is engineered under that:
 - prepare_only + trigger_dma (emission only on Pool; drains ride
   SWDGE queues 1..3 concurrently). Consumers gate on the prep's own
   completion sem (Tile's DMASW tick for preps is a pre-bump with no
   data sync).
 - m=25 positions per gather (3200 idxs) amortizes the ~1us fixed +
   ~1.4us Tile pre-bump + trigger per chunk. single_packet=False
   (single_packet caps at 64 descs/engine = m<=5... minus sem desc).
 - bf16 quads (1536B/lookup) keep the 4x-inflated reads at ~39MB/core,
   under the ~54us/chunk Pool emission time across 16 SDMA engines.
 - softmax tail: ACT exp (bf16 -> f32), DVE reduce/recip/scale, HWDGE
   store. Only the logits are bf16-quantized: ~1e-2 rel err vs 2e-2.
"""

import sys

import numpy as np

sys.path.insert(0, "/opt/trn_rl_repo")

N_POI = 100000
N_BINS = 168
DP = 192  # padded row length in bf16 elems (384B)
NQ = N_POI // 4  # quad rows
BATCH = 1024
SEQ = 200
N_CORES = 8
BPC = BATCH // N_CORES  # batch rows per core = 128 partitions

M = 25  # seq positions per quad dma_gather op

_NC_CACHE = {}


def build(seq=SEQ, m=M, nqueues=4, scratch=65536, tbufs=2, pbufs=2):
    """Build the per-core Bass program (SPMD: same NEFF on all cores)."""
    import concourse.bacc as bacc
    import concourse.tile as tile
    from concourse import bass, mybir

    assert seq % m == 0
    nch = seq // m
    nidx = BPC * m
    nc = bacc.Bacc(
        "TRN2",
        target_bir_lowering=False,
        debug=False,
        enable_asserts=False,
        num_devices=N_CORES,
        num_swdge_queues=nqueues,
        dynamic_dma_scratch_size=scratch,
        enable_partition_id=False,
    )
    qtab = nc.dram_tensor(
        "qtab", [NQ, 4 * DP], mybir.dt.bfloat16, kind="ExternalInput"
    ).ap()
    widx = nc.dram_tensor(
        "widx", [128, seq * 8], mybir.dt.int16, kind="ExternalInput"
    ).ap()
    msk = nc.dram_tensor(
        "msk", [BPC, 3 * seq], mybir.dt.uint8, kind="ExternalInput"
    ).ap()
    out = nc.dram_tensor(
        "out", [BPC, seq, N_BINS], mybir.dt.float32, kind="ExternalOutput"
    ).ap()

    with tile.TileContext(nc) as tc:
        with tc.tile_pool(name="const", bufs=1) as cpool, tc.tile_pool(
            name="quad", bufs=tbufs
        ) as tpool, tc.tile_pool(name="prob", bufs=pbufs) as ppool, tc.tile_pool(
            name="small", bufs=8
        ) as smpool:
            wt = cpool.tile([128, seq * 8], mybir.dt.int16)
            nc.sync.dma_start(out=wt[:], in_=widx[:])
            mt = cpool.tile([BPC, 3 * seq], mybir.dt.uint8)
            nc.sync.dma_start(out=mt[:], in_=msk[:])
            m3 = mt[:].rearrange("p (q s) -> p q s", q=3)

            for c in range(nch):
                T = tpool.tile([BPC, m * 4 * DP], mybir.dt.bfloat16, tag="T")
                T4 = T[:].rearrange("p (m q d) -> p m q d", m=m, q=4)
                q = 1 + c % (nqueues - 1)
                gsem = nc.alloc_semaphore(f"gsem{c}")
                nc.gpsimd.dma_gather(
                    out_ap=T[:].rearrange("p (m d) -> p m d", m=m),
                    in_ap=qtab[:],
                    idxs_ap=wt[:, c * m * 8 : (c + 1) * m * 8],
                    num_idxs=nidx,
                    num_idxs_reg=nidx,
                    elem_size=4 * DP,
                    elem_step=4 * DP,
                    single_packet=False,
                    prepare_only=True,
                    sem=gsem,
                    queue_num=q,
                )
                nc.gpsimd.trigger_dma(count=None, queue_num=q)
                # prep data flow is user-synced: gate the select on the
                # prep's completion sem (Tile's DMASW tick for preps is a
                # pre-bump with no data sync)
                nc.vector.wait_ge(gsem, 16)
                sel = T4[:, :, 0, :N_BINS]
                for qq in (1, 2, 3):
                    nc.vector.copy_predicated(
                        out=sel,
                        mask=m3[:, qq - 1, c * m : (c + 1) * m].to_broadcast(
                            [BPC, m, N_BINS]
                        ),
                        data=T4[:, :, qq, :N_BINS],
                    )
                P = ppool.tile([BPC, m * N_BINS], mybir.dt.float32, tag="P")
                P3 = P[:].rearrange("p (m d) -> p m d", m=m)
                nc.scalar.activation(
                    out=P3, in_=sel, func=mybir.ActivationFunctionType.Exp
                )
                sums = smpool.tile([BPC, m], mybir.dt.float32, tag="sums")
                nc.vector.tensor_reduce(
                    out=sums[:],
                    in_=P3,
                    axis=mybir.AxisListType.X,
                    op=mybir.AluOpType.add,
                )
                rec = smpool.tile([BPC, m], mybir.dt.float32, tag="rec")
                nc.vector.reciprocal(out=rec[:], in_=sums[:])
                nc.vector.tensor_tensor(
                    out=P3,
                    in0=P3,
                    in1=rec[:].to_broadcast([BPC, m, N_BINS]),
                    op=mybir.AluOpType.mult,
                )
                nc.sync.dma_start(out=out[:, c * m : (c + 1) * m, :], in_=P[:])
    nc.compile()
    return nc


def _prep_inputs(wekn, table, seq=SEQ, m=M):
    """Host-side layout/index prep: bf16 cast, padded quad table, wrapped
    int16 quad ids, sub-row masks, per-core shards."""
    import ml_dtypes

    tb = table.astype(ml_dtypes.bfloat16)
    qt = np.zeros((NQ, 4, DP), dtype=ml_dtypes.bfloat16)
    qt[:, :, :N_BINS] = tb.reshape(NQ, 4, N_BINS)
    qt = np.ascontiguousarray(qt.reshape(NQ, 4 * DP))
    nch = seq // m
    in_maps = []
    for core in range(N_CORES):
        wc = wekn[core * BPC : (core + 1) * BPC]
        quad = (wc // 4).astype(np.int16)
        sub = wc % 4
        wi = np.empty((16, seq * 8), dtype=np.int16)
        for c in range(nch):
            walk = quad[:, c * m : (c + 1) * m].T.reshape(-1)
            wi[:, c * m * 8 : (c + 1) * m * 8] = walk.reshape(m * 8, 16).T
        mk = np.empty((BPC, 3, seq), dtype=np.uint8)
        for qq in (1, 2, 3):
            mk[:, qq - 1] = (sub == qq).astype(np.uint8)
        in_maps.append(
            {
                "qtab": qt,
                "widx": np.tile(wi, (8, 1)),
                "msk": np.ascontiguousarray(mk.reshape(BPC, 3 * seq)),
            }
        )
    return in_maps


def _get_nc():
    if "nc" not in _NC_CACHE:
        _NC_CACHE["nc"] = build()
    return _NC_CACHE["nc"]


def kernel(**inputs) -> np.ndarray:
    wekn = np.asarray(inputs["inputs_wekn"]).astype(np.int64)
    table = np.ascontiguousarray(
        np.asarray(inputs["poi_freq_matrix"], dtype=np.float32)
    )
    assert wekn.shape == (BATCH, SEQ) and table.shape == (N_POI, N_BINS)

    from concourse.bass_utils import run_bass_kernel_spmd

    nc = _get_nc()
    in_maps = _prep_inputs(wekn, table)
    res = run_bass_kernel_spmd(nc, in_maps, core_ids=list(range(N_CORES)))
    return np.concatenate([res.results[c]["out"] for c in range(N_CORES)], axis=0)


if __name__ == "__main__":
    rng = np.random.default_rng(0)
    inputs = {
        "venueid2coor": rng.random((N_POI, 2), dtype=np.float32),
        "inputs_wekn": rng.integers(0, N_POI, size=(BATCH, SEQ), dtype=np.int64),
        "poi_freq_matrix": rng.standard_normal((N_POI, N_BINS), dtype=np.float32),
    }
    out = kernel(**inputs)
    print(out.shape, out.dtype)
